# revision 3
# baseline (speedup 1.0000x reference)
"""Causal single-head attention on 8 trn2 NeuronCores.

Problem (hardcoded): x [256,256,384] f32, Wq/Wk/Wv [384,64] f32
  q,k,v = x@W;  S = q@k^T * 384**-0.5; causal softmax; out = P@v  [256,256,64]

Sharding: data-parallel over batch B=256 -> 32 batches per core; weights
replicated. Per batch (T=256 tokens, C=384, H=64), per core:

  1. DMA x_b [256,384] as two [128,384] tiles (t-chunks).
  2. PE-transpose (fp32, exact) 6 128x128 blocks -> x^T [384c, 256t] in SBUF
     (rounded to f32r by the PSUM->SBUF evacuation copies).
  3. kT/qT = Wk^T@x^T, Wq^T@x^T   [64,256] each (f32r matmuls, N=256)
     vT = Wv^T@x^T [64,256]; append ones row -> v'T [65,256]; PE-transpose to
     v' [128,65] per t-chunk (v natural + ones column).
  4. S^T[j,i] per j-chunk: lhsT=kT chunk, rhs=qT  -> [128,256] PSUM.
     P^T = exp(scale*S^T) via ACT (PSUM->SBUF, f32r), then causal mask:
     multiplicative 0/1 upper-triangular 128x128 tile (+ zeroing the
     all-masked left half of chunk 1). No max-subtraction: |scale*S| <~ 3.
  5. O'^T [65,256] = sum_j v'[j,:]^T... accumulated over both j-chunks.
     Row 64 = softmax denominators (ones row of v').
  6. PE-transpose O'^T back to [128,65] per t-chunk; normalize cols 0:64 by
     reciprocal of col 64; DMA out.
"""
import numpy as np

N_CORES = 8
B, T, C, H = 256, 256, 384, 64
NB = B // N_CORES          # 32 batches per core
SCALE = float(C) ** -0.5

_state = {}


def _build():
    import concourse.bacc as bacc
    import concourse.tile as tile
    import concourse.mybir as mybir
    from concourse.masks import make_identity, make_upper_triangular

    dt = mybir.dt
    f32 = dt.float32
    f32r = dt.float32r
    AF = mybir.ActivationFunctionType

    nc = bacc.Bacc("TRN2", target_bir_lowering=False)
    x_d = nc.dram_tensor("x", [NB, T, C], f32, kind="ExternalInput")
    wq_d = nc.dram_tensor("Wq", [C, H], f32, kind="ExternalInput")
    wk_d = nc.dram_tensor("Wk", [C, H], f32, kind="ExternalInput")
    wv_d = nc.dram_tensor("Wv", [C, H], f32, kind="ExternalInput")
    out_d = nc.dram_tensor("out", [NB, T, H], f32, kind="ExternalOutput")

    with tile.TileContext(nc) as tc:
        with tc.tile_pool(name="setup", bufs=1) as setup, \
             tc.tile_pool(name="xin", bufs=3) as xin, \
             tc.tile_pool(name="work", bufs=2) as work, \
             tc.tile_pool(name="ps", bufs=1, space="PSUM") as ps:

            # --- one-time setup ---
            ident = setup.tile([128, 128], f32)
            make_identity(nc, ident)
            mask_st = setup.tile([128, 128], f32)
            make_upper_triangular(nc, mask_st, val=1.0, diag=True)
            mask = setup.tile([128, 128], f32r)
            nc.vector.tensor_copy(mask, mask_st)

            w_stage = setup.tile([128, 3 * C // 128 * 0 + 576], f32)  # [128, 576]
            # cc-chunk cc occupies cols cc*192:(cc+1)*192 as [Wk|Wq|Wv]
            for cc in range(3):
                nc.sync.dma_start(out=w_stage[:, cc * 192 + 0: cc * 192 + 64],
                                  in_=wk_d[cc * 128:(cc + 1) * 128, :])
                nc.sync.dma_start(out=w_stage[:, cc * 192 + 64: cc * 192 + 128],
                                  in_=wq_d[cc * 128:(cc + 1) * 128, :])
                nc.sync.dma_start(out=w_stage[:, cc * 192 + 128: cc * 192 + 192],
                                  in_=wv_d[cc * 128:(cc + 1) * 128, :])
            w_all = setup.tile([128, 576], f32r)
            nc.vector.tensor_copy(w_all, w_stage)  # round to f32r

            def wslice(cc, which):  # which: 0=k 1=q 2=v
                lo = cc * 192 + which * 64
                return w_all[:, lo:lo + 64]

            # --- per-batch pipeline ---
            for b in range(NB):
                x0 = xin.tile([128, C], f32)
                x1 = xin.tile([128, C], f32)
                nc.sync.dma_start(out=x0, in_=x_d[b, 0:128, :])
                nc.sync.dma_start(out=x1, in_=x_d[b, 128:256, :])

                # transpose x -> x^T  (xtps_a holds cc0+cc1, xtps_b holds cc2)
                xtps_a = ps.tile([128, 512], f32)
                xtps_b = ps.tile([128, 256], f32)
                for cc in range(3):
                    dst = xtps_a if cc < 2 else xtps_b
                    base = (cc % 2) * 256 if cc < 2 else 0
                    nc.tensor.transpose(dst[:, base:base + 128],
                                        x0[:, cc * 128:(cc + 1) * 128], ident)
                    nc.tensor.transpose(dst[:, base + 128:base + 256],
                                        x1[:, cc * 128:(cc + 1) * 128], ident)
                xt = work.tile([128, 768], f32r)
                nc.scalar.copy(xt[:, 0:512], xtps_a)
                nc.vector.tensor_copy(xt[:, 512:768], xtps_b)

                def xts(cc):
                    return xt[:, cc * 256:(cc + 1) * 256]

                # kT / qT  -> one PSUM bank [64, 512]
                kqps = ps.tile([64, 512], f32)
                for cc in range(3):
                    nc.tensor.matmul(kqps[:, 0:256], wslice(cc, 0), xts(cc),
                                     start=(cc == 0), stop=(cc == 2))
                for cc in range(3):
                    nc.tensor.matmul(kqps[:, 256:512], wslice(cc, 1), xts(cc),
                                     start=(cc == 0), stop=(cc == 2))
                kq_k = work.tile([64, 256], f32r)
                kq_q = work.tile([64, 256], f32r)
                nc.vector.tensor_copy(kq_k, kqps[:, 0:256])
                nc.scalar.copy(kq_q, kqps[:, 256:512])

                # vT [64,256] -> v'T [65,256] (ones row) -> v' [128,65] per tc
                vtps = ps.tile([64, 256], f32)
                for cc in range(3):
                    nc.tensor.matmul(vtps, wslice(cc, 2), xts(cc),
                                     start=(cc == 0), stop=(cc == 2))
                vtp = work.tile([65, 256], f32)
                nc.scalar.copy(vtp[0:64, :], vtps)
                nc.gpsimd.memset(vtp[64:65, :], 1.0)
                vpps = ps.tile([128, 130], f32)
                vp0 = work.tile([128, 65], f32r)
                vp1 = work.tile([128, 65], f32r)
                nc.tensor.transpose(vpps[:, 0:65], vtp[:, 0:128],
                                    ident[0:65, 0:65])
                nc.tensor.transpose(vpps[:, 65:130], vtp[:, 128:256],
                                    ident[0:65, 0:65])
                nc.vector.tensor_copy(vp0, vpps[:, 0:65])
                nc.vector.tensor_copy(vp1, vpps[:, 65:130])

                # S^T per j-chunk + exp + causal mask
                stps = ps.tile([128, 512], f32)
                nc.tensor.matmul(stps[:, 0:256], kq_k[:, 0:128], kq_q,
                                 start=True, stop=True)
                nc.tensor.matmul(stps[:, 256:512], kq_k[:, 128:256], kq_q,
                                 start=True, stop=True)
                pt0 = work.tile([128, 256], f32r)
                pt1 = work.tile([128, 128], f32r)
                nc.scalar.activation(pt0, stps[:, 0:256], AF.Exp, scale=SCALE)
                # chunk-1 rows attend only to keys j>=128 -> cols 128:256
                nc.scalar.activation(pt1, stps[:, 384:512], AF.Exp, scale=SCALE)
                nc.vector.tensor_mul(pt0[:, 0:128], pt0[:, 0:128], mask)
                nc.vector.tensor_mul(pt1, pt1, mask)

                # O'^T [65,256] accumulate over j-chunks (chunk 1 only touches
                # output cols 128:256; cols 0:128 get no chunk-1 contribution)
                ops = ps.tile([65, 256], f32)
                nc.tensor.matmul(ops, vp0, pt0, start=True, stop=False)
                nc.tensor.matmul(ops[:, 128:256], vp1, pt1,
                                 start=False, stop=True)
                ot = work.tile([65, 256], f32)
                nc.scalar.copy(ot, ops)

                # transpose back, normalize, store
                ofps = ps.tile([128, 130], f32)
                nc.tensor.transpose(ofps[:, 0:65], ot[:, 0:128],
                                    ident[0:65, 0:65])
                nc.tensor.transpose(ofps[:, 65:130], ot[:, 128:256],
                                    ident[0:65, 0:65])
                rec0 = work.tile([128, 1], f32)
                rec1 = work.tile([128, 1], f32)
                nc.vector.reciprocal(rec0, ofps[:, 64:65])
                nc.vector.reciprocal(rec1, ofps[:, 129:130])
                oo0 = work.tile([128, 64], f32)
                oo1 = work.tile([128, 64], f32)
                nc.vector.tensor_scalar_mul(oo0, ofps[:, 0:64], rec0)
                nc.scalar.mul(oo1, ofps[:, 65:129], rec1)
                nc.sync.dma_start(out=out_d[b, 0:128, :], in_=oo0)
                nc.sync.dma_start(out=out_d[b, 128:256, :], in_=oo1)

    nc.finalize()
    return nc


def kernel(x, Wq, Wk, Wv, _trace=False):
    from concourse.bass_utils import run_bass_kernel_spmd

    if "nc" not in _state:
        _state["nc"] = _build()
    nc = _state["nc"]

    x = np.ascontiguousarray(np.asarray(x, dtype=np.float32))
    wq = np.ascontiguousarray(np.asarray(Wq, dtype=np.float32))
    wk = np.ascontiguousarray(np.asarray(Wk, dtype=np.float32))
    wv = np.ascontiguousarray(np.asarray(Wv, dtype=np.float32))

    in_maps = [
        {"x": x[i * NB:(i + 1) * NB], "Wq": wq, "Wk": wk, "Wv": wv}
        for i in range(N_CORES)
    ]
    res = run_bass_kernel_spmd(nc, in_maps, core_ids=list(range(N_CORES)),
                               trace=_trace)
    _state["exec_time_ns"] = res.exec_time_ns
    _state["trace"] = res.instructions_and_trace
    return np.concatenate([res.results[i]["out"] for i in range(N_CORES)],
                          axis=0)
